# revision 26
# baseline (speedup 1.0000x reference)
"""Chamfer loss kernel for 8 Trainium2 NeuronCores.

Problem: pred [16, 2048, 3] f32, gt [16, 2048, 3] f32 ->
  loss = mean_n(min_m |pred_n - gt_m|^2) + mean_m(min_n |pred_n - gt_m|^2)  (scalar f32)

Sharding: data-parallel over batch B=16 -> 2 batches per core on 8 cores.

Per-core algorithm (per batch):
  d[n, m] = |a_n|^2 + |b_m|^2 - 2 a_n . b_m  is computed with ONE bf16 matmul
  per [128, 512] tile using a K=24 augmented contraction: every f32 operand is
  split into 3 bf16 terms (hi/lo/lo2); the 6 dominant cross products per
  coordinate plus 3+3 norm rows against ones reproduce the f32 product to
  ~2^-24 relative (bf16 products are exact in the PE's fp22 pipeline).

  Each [128, 2048] f32 PSUM strip (row-tile t) is drained by ScalarE to fp16
  in SBUF, 4 strips per group tile. VectorE then:
    - folds each group 2048->64 with tensor_tensor(min) ops (2x 16-bit mode)
      and one 1x tensor_reduce(min) -> exact rowmins,
    - maintains a column-min accumulator acc = min(acc, strip).
  The accumulator is shipped to the host, which finishes the cross-partition
  column min (removing the on-device transpose+reduce tail) and computes the
  means in f64.

  The two batches are emitted group-interleaved so the PE never stalls on a
  batch boundary, and the first weight blocks are DMAd ahead of the rest so
  strip 0 of both batches can start immediately.
"""

import sys

import numpy as np

sys.path.insert(0, "/opt/trn_rl_repo")

import ml_dtypes  # noqa: E402

B, N, M, D = 16, 2048, 2048, 3
NCORES = 8
NB = B // NCORES          # batches per core
K = 24                    # augmented contraction rows
NT = N // 128             # row tiles per batch
MC = 512                  # matmul moving chunk (one PSUM bank of f32)
G = 4                     # strips per drain group

_BF16 = ml_dtypes.bfloat16

_compiled = {}


def _split3(x32: np.ndarray):
    """Split f32 array into three bf16 terms whose sum reproduces x to ~2^-24."""
    x32 = x32.astype(np.float32)
    h = x32.astype(_BF16)
    r = x32 - h.astype(np.float32)
    l = r.astype(_BF16)
    q = (r - l.astype(np.float32)).astype(_BF16)
    return h, l, q


def _build_inputs(pred: np.ndarray, gt: np.ndarray):
    """Build augmented [B, K, N] bf16 operands W (pred side, lhsT) and
    V (gt side, rhs) such that sum_k W[k, n] V[k, m] ~= |a_n - b_m|^2."""
    W = np.zeros((B, K, N), dtype=_BF16)
    V = np.zeros((B, K, N), dtype=_BF16)

    ones = np.ones((B, N), dtype=_BF16)
    for c in range(D):
        a = -2.0 * pred[:, :, c].astype(np.float32)   # exact scale by -2
        b = gt[:, :, c].astype(np.float32)
        ah, al, aq = _split3(a)
        bh, bl, bq = _split3(b)
        r = 6 * c
        # (-2a) * b  ~=  ah*bh + ah*bl + al*bh + al*bl + ah*bq + aq*bh
        W[:, r + 0] = ah; V[:, r + 0] = bh
        W[:, r + 1] = ah; V[:, r + 1] = bl
        W[:, r + 2] = al; V[:, r + 2] = bh
        W[:, r + 3] = al; V[:, r + 3] = bl
        W[:, r + 4] = ah; V[:, r + 4] = bq
        W[:, r + 5] = aq; V[:, r + 5] = bh

    a2 = (pred.astype(np.float64) ** 2).sum(-1)
    b2 = (gt.astype(np.float64) ** 2).sum(-1)
    a2h, a2l, a2q = _split3(a2.astype(np.float32))
    b2h, b2l, b2q = _split3(b2.astype(np.float32))
    W[:, 18] = a2h; V[:, 18] = ones
    W[:, 19] = a2l; V[:, 19] = ones
    W[:, 20] = a2q; V[:, 20] = ones
    W[:, 21] = ones; V[:, 21] = b2h
    W[:, 22] = ones; V[:, 22] = b2l
    W[:, 23] = ones; V[:, 23] = b2q
    return W, V


def _build_nc(loop_r: int = 1):
    import concourse.mybir as mybir
    from concourse import bacc
    from concourse.tile import TileContext

    F32 = mybir.dt.float32
    F16 = mybir.dt.float16
    BF16 = mybir.dt.bfloat16
    MIN = mybir.AluOpType.min
    X = mybir.AxisListType.X

    nc = bacc.Bacc("TRN2")
    w_d = nc.dram_tensor("w", [NB, K, N], BF16, kind="ExternalInput")
    v_d = nc.dram_tensor("v", [NB, K, N], BF16, kind="ExternalInput")
    outr_d = nc.dram_tensor("outr", [NB, 128, NT], F16, kind="ExternalOutput")
    # partial colmin accumulator, one slot per strip-within-group; the host
    # finishes the cross-slot and cross-partition min
    outa_d = nc.dram_tensor("outa", [NB, G, 128, N], F16,
                            kind="ExternalOutput")

    H2 = N // 2

    with TileContext(nc) as tc:
        with tc.tile_pool(name="io", bufs=2) as iop, \
             tc.tile_pool(name="dst", bufs=5) as dstp, \
             tc.tile_pool(name="fold", bufs=2) as foldp, \
             tc.tile_pool(name="accd", bufs=2) as accdp, \
             tc.tile_pool(name="res", bufs=2) as resp, \
             tc.tile_pool(name="ps", bufs=2, space="PSUM") as psp:

            def body(_i=None):
                # Both batches' first blocks load before anything else (the
                # group-interleaved loop needs batch 1's weights right away);
                # the rest follows in matmul consumption order.
                # DMA order = first-consumption order: strip (t=0, b) needs
                # ALL of v_sb[b] plus only the first 128 columns of w_sb[b].
                wv = []
                for b in range(NB):
                    w_sb = iop.tile([K, N], BF16, tag="w", name=f"w_sb{b}")
                    v_sb = iop.tile([K, N], BF16, tag="v", name=f"v_sb{b}")
                    nc.sync.dma_start(out=v_sb[:, :H2], in_=v_d[b][:, :H2])
                    nc.sync.dma_start(out=v_sb[:, H2:], in_=v_d[b][:, H2:])
                    nc.sync.dma_start(out=w_sb[:, :MC], in_=w_d[b][:, :MC])
                    wv.append((w_sb, v_sb))
                for b in range(NB):
                    w_sb, v_sb = wv[b]
                    nc.sync.dma_start(out=w_sb[:, MC:H2], in_=w_d[b][:, MC:H2])
                    nc.sync.dma_start(out=w_sb[:, H2:], in_=w_d[b][:, H2:])

                rmin = []
                acc = []
                prev_dsb = [None] * NB
                for b in range(NB):
                    rmin.append(resp.tile([128, NT], F16, tag="rmin",
                                          name=f"rmin{b}"))
                    acc.append(accdp.tile([128, G, N], F16, tag="accd",
                                          name=f"acc{b}"))

                for g in range(NT // G):
                    for b in range(NB):
                        w_sb, v_sb = wv[b]
                        dsb = dstp.tile([128, G, N], F16, tag="dsb",
                                        name="dsb")
                        f0 = foldp.tile([128, G, H2], F16, tag="f0",
                                        name="f0")
                        for j in range(G):
                            t = g * G + j
                            ps = psp.tile([128, N], F32, tag="ps", name="ps")
                            for c in range(N // MC):
                                nc.tensor.matmul(
                                    ps[:, c * MC:(c + 1) * MC],
                                    lhsT=w_sb[:, t * 128:(t + 1) * 128],
                                    rhs=v_sb[:, c * MC:(c + 1) * MC],
                                    start=True, stop=True,
                                )
                            # ScalarE drain: PSUM f32 -> SBUF fp16
                            nc.scalar.copy(dsb[:, j, :], ps)
                            # per-strip fold level 0 keeps DVE fed from the
                            # first drain on (the group-wide ops below can
                            # only start once the whole group is drained)
                            nc.vector.tensor_tensor(
                                f0[:, j, :], dsb[:, j, :H2], dsb[:, j, H2:],
                                op=MIN)

                        # colmin at group granularity (VectorE 2x 16-bit),
                        # one TT over all 4 strip slots; g==1 initializes
                        # acc = min(group0, group1) in one op. The last
                        # group goes slot-wise so each slot's outa DMA (2MB
                        # total per batch) starts as early as possible.
                        if g == 1:
                            nc.vector.tensor_tensor(
                                acc[b], prev_dsb[b], dsb, op=MIN)
                        elif g == NT // G - 1:
                            for j in range(G):
                                nc.vector.tensor_tensor(
                                    acc[b][:, j, :], acc[b][:, j, :],
                                    dsb[:, j, :], op=MIN)
                                nc.sync.dma_start(out=outa_d[b][j],
                                                  in_=acc[b][:, j, :])
                        elif g > 1:
                            nc.vector.tensor_tensor(
                                acc[b], acc[b], dsb, op=MIN)
                        prev_dsb[b] = dsb

                        # remaining rowmin fold levels: 1024 -> 32 at 2x,
                        # then one 1x reduce
                        f = f0
                        w = H2
                        lvl = 1
                        while w > 32:
                            w //= 2
                            nf = foldp.tile([128, G, w], F16, tag=f"f{lvl}",
                                            name="nf")
                            nc.vector.tensor_tensor(
                                nf, f[:, :, :w], f[:, :, w:], op=MIN)
                            f = nf
                            lvl += 1
                        nc.vector.tensor_reduce(
                            rmin[b][:, g * G:(g + 1) * G], f, axis=X, op=MIN)

                for b in range(NB):
                    nc.sync.dma_start(out=outr_d[b], in_=rmin[b])

            if loop_r > 1:
                with tc.For_i(0, loop_r, 1) as i:
                    body(i)
            else:
                body()
    nc.finalize()
    return nc


def _get_nc():
    if "nc" not in _compiled:
        _compiled["nc"] = _build_nc()
    return _compiled["nc"]


def kernel(pred, gt) -> np.ndarray:
    pred = np.asarray(pred, dtype=np.float32)
    gt = np.asarray(gt, dtype=np.float32)
    assert pred.shape == (B, N, D) and gt.shape == (B, M, D)

    from concourse.bass_utils import run_bass_kernel_spmd

    W, V = _build_inputs(pred, gt)
    in_maps = [
        {"w": np.ascontiguousarray(W[c * NB:(c + 1) * NB]),
         "v": np.ascontiguousarray(V[c * NB:(c + 1) * NB])}
        for c in range(NCORES)
    ]
    nc = _get_nc()
    res = None
    err = None
    for _attempt in range(3):
        try:
            res = run_bass_kernel_spmd(nc, in_maps, list(range(NCORES)))
            break
        except Exception as e:  # rare transient NRT device errors; retry
            err = e
    if res is None:
        raise err

    tot1 = 0.0
    tot2 = 0.0
    for c in range(NCORES):
        tot1 += np.asarray(res.results[c]["outr"], dtype=np.float64).sum()
        acc = np.asarray(res.results[c]["outa"], dtype=np.float64)  # [NB,G,128,N]
        tot2 += acc.min(axis=(1, 2)).sum()
    loss = tot1 / (B * N) + tot2 / (B * M)
    return np.array(loss, dtype=np.float32)
